# revision 9
# baseline (speedup 1.0000x reference)
"""Trainium2 Bass kernel for nn_CausalSelfAttention (B=2, T=2048, C=2048, H=16).

Sharding: tensor-parallel over heads (2 heads/core on 8 cores).
 - QKV projection computed per-core for its heads only (x replicated).
 - RMSNorm over full C needs cross-head sums of squares -> tiny AllReduce.
 - Attention fully local per (batch, head) with a transposed, max-free softmax
   (scores bounded, so exp(tau*S - 10) never overflows; shift cancels).
 - Output projection re-partitions y from head(channel)-sharding to
   token-row sharding via AllToAll, then each core computes its 512 token
   rows against the full W_proj; host concatenates rows.

Matmuls run in float32r (fp32 rounded to 11-bit mantissa, full PE rate).
"""

import math

import numpy as np

import concourse.bass as bass
import concourse.mybir as mybir
import concourse.tile as tile
from concourse import bacc
from concourse.bass_utils import run_bass_kernel_spmd

N_CORES = 8
CORE_IDS = list(range(N_CORES))
B, T, C = 2, 2048, 2048
H, HD = 16, 128
HPC = H // N_CORES          # heads per core = 2
CPC = HPC * HD              # channels per core = 256
BT = B * T                  # 4096 token rows, batch-major
TOKS = BT // N_CORES        # 512 token rows per core for the projection
TC = 256                    # token-chunk for the QKV phase
TAU = 1.0 / math.sqrt(HD)
EPS = 1e-6
EXP_SHIFT = -10.0
MASK_VAL = -1e30

f32 = mybir.dt.float32
f32r = mybir.dt.float32r


def _round_f32r(x: np.ndarray) -> np.ndarray:
    """Round fp32 to fp32r (11-bit mantissa, round-to-nearest-even)."""
    b = np.ascontiguousarray(x, dtype=np.float32).view(np.uint32)
    out = (b + np.uint32(0x7FF) + ((b >> np.uint32(12)) & np.uint32(1))) & np.uint32(
        0xFFFFF000
    )
    return out.view(np.float32)


def _rope_cos_sin():
    """cos/sin tables [T, HD], faithful to the reference construction."""
    i = np.arange(0, HD, 2, dtype=np.float32)
    pos = np.power(np.float32(10000.0), (-2.0 * i - 1.0).astype(np.float32) / np.float32(HD))
    token_seq = np.arange(T, dtype=np.float32)[:, None] * pos[None, :]
    ang = np.concatenate([token_seq, token_seq], axis=-1).astype(np.float32)
    return np.cos(ang), np.sin(ang)


def build_nc() -> bass.Bass:
    nc = bacc.Bacc(trn_type="TRN2", target_bir_lowering=False, num_devices=N_CORES)

    # ---- external inputs (per-core contents prepared on host) ----
    x_d = nc.dram_tensor("x", [BT, C], f32, kind="ExternalInput")
    wqk_d = nc.dram_tensor("wqk_t", [C, 4 * HD], f32r, kind="ExternalInput")
    wv_d = nc.dram_tensor("wv_t", [C, CPC], f32r, kind="ExternalInput")
    wp_d = nc.dram_tensor("wp_t", [C, C], f32r, kind="ExternalInput")
    ra_d = nc.dram_tensor("ropeA", [HPC, HD, T], f32, kind="ExternalInput")
    rs_d = nc.dram_tensor("ropeS", [HPC, HD, T], f32, kind="ExternalInput")
    rp_d = nc.dram_tensor("rperm", [HD, HD], f32r, kind="ExternalInput")
    id_d = nc.dram_tensor("ident", [128, 128], f32, kind="ExternalInput")
    oc_d = nc.dram_tensor("ones_col", [128, 1], f32r, kind="ExternalInput")
    or_d = nc.dram_tensor("ones_row", [1, 128], f32r, kind="ExternalInput")
    cm_d = nc.dram_tensor("cmask", [4, 128, 512], f32, kind="ExternalInput")

    out_d = nc.dram_tensor("out", [TOKS, C], f32, kind="ExternalOutput")

    # ---- internal DRAM ----
    ssq_in = nc.dram_tensor("ssq_in", [1, 2 * BT], f32)
    srow_d = nc.dram_tensor("srow_bounce", [1, 2 * BT], f32r)
    ssq_out = nc.dram_tensor("ssq_out", [1, 2 * BT], f32, addr_space="Shared")
    vtok = nc.dram_tensor("vtok", [HPC, BT, HD], f32r)
    a2a_in = nc.dram_tensor("a2a_in", [N_CORES, CPC, TOKS], f32r)
    a2a_out = nc.dram_tensor("a2a_out", [N_CORES, CPC, TOKS], f32r)

    NCH = C // 128  # 16 contraction chunks

    with tile.TileContext(nc) as tc:
        with tc.tile_pool(name="persist", bufs=1) as persist, \
             tc.tile_pool(name="consts", bufs=1) as consts:
            # persistent qT/kT storage: [q_h0, q_h1, k_h0, k_h1] x [128, BT]
            qk_sb = [persist.tile([128, BT], f32, tag=f"qk{i}", name=f"qk{i}") for i in range(4)]

            ident = consts.tile([128, 128], f32)
            nc.sync.dma_start(out=ident, in_=id_d[:, :])
            rperm = consts.tile([HD, HD], f32r)
            nc.sync.dma_start(out=rperm, in_=rp_d[:, :])
            ones_col = consts.tile([128, 1], f32r)
            nc.sync.dma_start(out=ones_col, in_=oc_d[:, :])
            ones_row = consts.tile([1, 128], f32r)
            nc.sync.dma_start(out=ones_row, in_=or_d[:, :])
            cmask = consts.tile([128, 4, 512], f32)
            nc.sync.dma_start(out=cmask, in_=cm_d.rearrange("r k t -> k r t"))
            eps_ap = consts.tile([128, 1], f32)
            nc.vector.memset(eps_ap, EPS)
            shift_ap = consts.tile([128, 1], f32)
            nc.vector.memset(shift_ap, EXP_SHIFT)

            # ================= QKV projection =================
            with tc.tile_pool(name="wqk", bufs=1) as wqkp, \
                 tc.tile_pool(name="wv", bufs=1) as wvp, \
                 tc.tile_pool(name="xin", bufs=3) as xinp, \
                 tc.tile_pool(name="xt", bufs=2) as xtp, \
                 tc.tile_pool(name="vev", bufs=3) as vevp, \
                 tc.tile_pool(name="ps_t", bufs=2, space="PSUM") as pst, \
                 tc.tile_pool(name="ps_mm", bufs=3, space="PSUM") as psmm:
                wqk_sb = wqkp.tile([128, NCH, 4 * HD], f32r)
                nc.sync.dma_start(out=wqk_sb, in_=wqk_d.rearrange("(c p) f -> p c f", p=128))
                wv_sb = wvp.tile([128, NCH, CPC], f32r)
                nc.sync.dma_start(out=wv_sb, in_=wv_d.rearrange("(c p) f -> p c f", p=128))

                for tcix in range(BT // TC):
                    r0 = tcix * TC
                    xt_t = xtp.tile([128, NCH, TC], f32r, tag="xt")
                    # load + transpose x chunk
                    for t2 in range(TC // 128):
                        x_t = xinp.tile([128, C], f32, tag="xin")
                        nc.sync.dma_start(out=x_t, in_=x_d[r0 + 128 * t2:r0 + 128 * (t2 + 1), :])
                        for cc in range(NCH):
                            ps = pst.tile([128, 128], f32, tag="tp")
                            nc.tensor.transpose(ps, x_t[:, 128 * cc:128 * (cc + 1)], ident)
                            nc.any.tensor_copy(
                                out=xt_t[:, cc, 128 * t2:128 * (t2 + 1)], in_=ps)
                    # q/k blocks -> transposed layout [ch, tok]
                    for chb in range(4):
                        ps = psmm.tile([128, TC], f32, tag="qk")
                        for cc in range(NCH):
                            nc.tensor.matmul(
                                ps,
                                wqk_sb[:, cc, 128 * chb:128 * (chb + 1)],
                                xt_t[:, cc, :],
                                start=(cc == 0), stop=(cc == NCH - 1))
                        nc.any.tensor_copy(
                            out=qk_sb[chb][:, r0:r0 + TC].bitcast(f32r), in_=ps)
                    # v block -> token-major, spilled to DRAM
                    for t2 in range(TC // 128):
                        ps = psmm.tile([128, CPC], f32, tag="v")
                        for cc in range(NCH):
                            nc.tensor.matmul(
                                ps,
                                xt_t[:, cc, 128 * t2:128 * (t2 + 1)],
                                wv_sb[:, cc, :],
                                start=(cc == 0), stop=(cc == NCH - 1))
                        vev = vevp.tile([128, CPC], f32r, tag="vev")
                        nc.any.tensor_copy(out=vev, in_=ps)
                        for h in range(HPC):
                            nc.sync.dma_start(
                                out=vtok[h, r0 + 128 * t2:r0 + 128 * (t2 + 1), :],
                                in_=vev[:, HD * h:HD * (h + 1)])

            # ================= partial sum-of-squares + AllReduce =================
            with tc.tile_pool(name="ssq", bufs=1) as ssqp, \
                 tc.tile_pool(name="sqt", bufs=3) as sqtp, \
                 tc.tile_pool(name="ps_ss", bufs=2, space="PSUM") as psss:
                ssq_sb = ssqp.tile([1, 2 * BT], f32)
                for c8 in range(BT // 512):
                    cols = slice(512 * c8, 512 * (c8 + 1))
                    for ti, (c0, c1) in enumerate(((0, 1), (2, 3))):
                        ps = psss.tile([1, 512], f32, tag="ss")
                        for k, chb in enumerate((c0, c1)):
                            sq = sqtp.tile([128, 512], f32r, tag="sq")
                            nc.vector.tensor_mul(
                                sq, qk_sb[chb][:, cols], qk_sb[chb][:, cols])
                            nc.tensor.matmul(ps, ones_col, sq,
                                             start=(k == 0), stop=(k == 1))
                        nc.scalar.copy(
                            out=ssq_sb[0:1, BT * ti + 512 * c8: BT * ti + 512 * (c8 + 1)],
                            in_=ps)
                nc.sync.dma_start(out=ssq_in[:, :], in_=ssq_sb)
                nc.gpsimd.collective_compute(
                    "AllReduce", mybir.AluOpType.add,
                    replica_groups=[CORE_IDS],
                    ins=[ssq_in[:, :]], outs=[ssq_out[:, :]])

            # ================= norm scales + RoPE =================
            with tc.tile_pool(name="nrm", bufs=1) as nrmp, \
                 tc.tile_pool(name="rope", bufs=1) as ropep, \
                 tc.tile_pool(name="rtmp", bufs=2) as rtmp, \
                 tc.tile_pool(name="ps_r", bufs=2, space="PSUM") as psr:
                FR = 2 * BT // 128
                sq2d = nrmp.tile([128, FR], f32)
                nc.sync.dma_start(
                    out=sq2d, in_=ssq_out.rearrange("a (p f) -> (a p) f", p=128))
                sd2d = nrmp.tile([128, FR], f32)
                nc.scalar.activation(out=sd2d, in_=sq2d,
                                     func=mybir.ActivationFunctionType.Sqrt,
                                     bias=eps_ap, scale=1.0 / C)
                rc2d = nrmp.tile([128, FR], f32r)
                with nc.allow_low_precision(reason="f32r rounding of norm scale is fine"):
                    nc.vector.reciprocal(out=rc2d, in_=sd2d)
                # reshape [128, FR] -> one row [1, 2*BT] via a DRAM bounce
                nc.sync.dma_start(
                    out=srow_d.rearrange("a (p f) -> (a p) f", p=128), in_=rc2d)
                s_row = nrmp.tile([1, 2 * BT], f32r)
                nc.sync.dma_start(out=s_row, in_=srow_d[:, :])
                # broadcast scales along partitions: s_bc[ti] is [128, BT]
                s_bc = [nrmp.tile([128, BT], f32, tag=f"sbc{ti}", name=f"sbc{ti}") for ti in range(2)]
                for ti in range(2):
                    for c8 in range(BT // 512):
                        ps = psr.tile([128, 512], f32, tag="bc")
                        nc.tensor.matmul(
                            ps, ones_row,
                            s_row[0:1, BT * ti + 512 * c8: BT * ti + 512 * (c8 + 1)],
                            start=True, stop=True)
                        nc.any.tensor_copy(
                            out=s_bc[ti][:, 512 * c8:512 * (c8 + 1)], in_=ps)

                ra_sb = ropep.tile([128, HPC, T], f32)
                nc.sync.dma_start(out=ra_sb, in_=ra_d.rearrange("h d t -> d h t"))
                rs_sb = ropep.tile([128, HPC, T], f32)
                nc.sync.dma_start(out=rs_sb, in_=rs_d.rearrange("h d t -> d h t"))

                for chb in range(4):
                    ti, h = chb // 2, chb % 2
                    for b in range(B):
                        for j4 in range(T // 512):
                            col = slice(b * T + 512 * j4, b * T + 512 * (j4 + 1))
                            tcol = slice(512 * j4, 512 * (j4 + 1))
                            ps = psr.tile([128, 512], f32, tag="rp")
                            nc.tensor.matmul(
                                ps, rperm, qk_sb[chb][:, col].bitcast(f32r),
                                start=True, stop=True)
                            t1 = rtmp.tile([128, 512], f32, tag="t1")
                            nc.vector.tensor_mul(t1, qk_sb[chb][:, col], ra_sb[:, h, tcol])
                            t2 = rtmp.tile([128, 512], f32, tag="t2")
                            nc.vector.tensor_mul(t2, ps, rs_sb[:, h, tcol])
                            nc.vector.tensor_add(t1, t1, t2)
                            nc.vector.tensor_mul(
                                out=qk_sb[chb][:, col].bitcast(f32r),
                                in0=t1, in1=s_bc[ti][:, col])

            # ================= attention =================
            with tc.tile_pool(name="yt", bufs=1) as ytp, \
                 tc.tile_pool(name="vbh", bufs=2) as vbhp, \
                 tc.tile_pool(name="pt", bufs=3) as ptp, \
                 tc.tile_pool(name="att_sm", bufs=2) as attsm, \
                 tc.tile_pool(name="ps_s", bufs=3, space="PSUM") as pss, \
                 tc.tile_pool(name="ps_y", bufs=2, space="PSUM") as psy, \
                 tc.tile_pool(name="ps_d", bufs=2, space="PSUM") as psd:
                yt_sb = [ytp.tile([128, BT], f32, tag=f"yt{h}", name=f"yt{h}") for h in range(HPC)]
                for h in range(HPC):
                    for b in range(B):
                        vb = vbhp.tile([128, T // 128, HD], f32r, tag="vbh")
                        nc.sync.dma_start(
                            out=vb,
                            in_=vtok[h, b * T:(b + 1) * T, :].rearrange(
                                "(t p) d -> p t d", p=128))
                        for g in range(T // 512):
                            qcol = slice(b * T + 512 * g, b * T + 512 * (g + 1))
                            ps_yt = psy.tile([128, 512], f32, tag="y")
                            ps_dt = psd.tile([1, 512], f32, tag="d")
                            nj = 4 * g + 4
                            for j in range(nj):
                                kcol = slice(b * T + 128 * j, b * T + 128 * (j + 1))
                                ps_st = pss.tile([128, 512], f32, tag="s")
                                nc.tensor.matmul(
                                    ps_st,
                                    qk_sb[2 + h][:, kcol].bitcast(f32r),
                                    qk_sb[h][:, qcol].bitcast(f32r),
                                    start=True, stop=True)
                                r = j - 4 * g
                                if r >= 0:
                                    nc.vector.tensor_add(ps_st, ps_st, cmask[:, r, :])
                                pt = ptp.tile([128, 512], f32r, tag="pt")
                                nc.scalar.activation(
                                    out=pt, in_=ps_st,
                                    func=mybir.ActivationFunctionType.Exp,
                                    bias=shift_ap, scale=TAU)
                                nc.tensor.matmul(ps_dt, ones_col, pt,
                                                 start=(j == 0), stop=(j == nj - 1))
                                nc.tensor.matmul(ps_yt, vb[:, j, :], pt,
                                                 start=(j == 0), stop=(j == nj - 1))
                            d_sb = attsm.tile([1, 512], f32, tag="dsb")
                            nc.scalar.copy(out=d_sb, in_=ps_dt)
                            r_sb = attsm.tile([1, 512], f32r, tag="rsb")
                            with nc.allow_low_precision(reason="f32r rounding of softmax denom is fine"):
                                nc.vector.reciprocal(out=r_sb, in_=d_sb)
                            ps_bt = pss.tile([128, 512], f32, tag="s")
                            nc.tensor.matmul(ps_bt, ones_row, r_sb[0:1, :],
                                             start=True, stop=True)
                            rb = attsm.tile([128, 512], f32, tag="rb")
                            nc.any.tensor_copy(out=rb, in_=ps_bt)
                            nc.vector.tensor_mul(
                                out=yt_sb[h][:, qcol].bitcast(f32r),
                                in0=ps_yt, in1=rb)

                # stage AllToAll input: peer p gets my channels for its tokens
                for p in range(N_CORES):
                    for h in range(HPC):
                        nc.sync.dma_start(
                            out=a2a_in[p, HD * h:HD * (h + 1), :],
                            in_=yt_sb[h][:, TOKS * p:TOKS * (p + 1)].bitcast(f32r))

            nc.gpsimd.collective_compute(
                "AllToAll", mybir.AluOpType.bypass,
                replica_groups=[CORE_IDS],
                ins=[a2a_in[:, :, :]], outs=[a2a_out[:, :, :]])

            # ================= output projection =================
            with tc.tile_pool(name="yf", bufs=1) as yfp, \
                 tc.tile_pool(name="wp", bufs=3) as wpp, \
                 tc.tile_pool(name="oev", bufs=3) as oevp, \
                 tc.tile_pool(name="ps_o", bufs=1, space="PSUM") as pso:
                yf_sb = yfp.tile([128, NCH, TOKS], f32r)
                nc.sync.dma_start(
                    out=yf_sb,
                    in_=a2a_out.rearrange("c q t -> (c q) t").rearrange(
                        "(cc p) t -> p cc t", p=128))
                for o4 in range(C // 512):
                    ps_list = [pso.tile([128, 512], f32, tag=f"o{t4}", name=f"o{o4}_{t4}")
                               for t4 in range(TOKS // 128)]
                    for cc in range(NCH):
                        wp_sb = wpp.tile([128, 512], f32r, tag="wp")
                        nc.sync.dma_start(
                            out=wp_sb,
                            in_=wp_d[128 * cc:128 * (cc + 1), 512 * o4:512 * (o4 + 1)])
                        for t4 in range(TOKS // 128):
                            nc.tensor.matmul(
                                ps_list[t4],
                                yf_sb[:, cc, 128 * t4:128 * (t4 + 1)],
                                wp_sb,
                                start=(cc == 0), stop=(cc == NCH - 1))
                    for t4 in range(TOKS // 128):
                        o_sb = oevp.tile([128, 512], f32, tag="osb")
                        nc.any.tensor_copy(out=o_sb, in_=ps_list[t4])
                        nc.sync.dma_start(
                            out=out_d[128 * t4:128 * (t4 + 1), 512 * o4:512 * (o4 + 1)],
                            in_=o_sb)

    nc.finalize()
    return nc


_NC_CACHE = None


def prepare_in_maps(x: np.ndarray, W_qkv: np.ndarray, W_proj: np.ndarray,
                    g_qk: np.ndarray) -> list:
    x = np.asarray(x, dtype=np.float32)
    W_qkv = np.asarray(W_qkv, dtype=np.float32)
    W_proj = np.asarray(W_proj, dtype=np.float32)
    g_qk = np.asarray(g_qk, dtype=np.float32)

    x_flat = np.ascontiguousarray(x.reshape(BT, C))
    wp_t = _round_f32r(W_proj.T)

    cos, sin = _rope_cos_sin()          # [T, HD]
    cosT, sinT = cos.T, sin.T           # [HD, T]

    ident = np.eye(128, dtype=np.float32)
    rperm = np.zeros((HD, HD), dtype=np.float32)
    for d in range(HD):
        rperm[d, d ^ 1] = 1.0
    ones_col = np.ones((128, 1), dtype=np.float32)
    ones_row = np.ones((1, 128), dtype=np.float32)

    cmask = np.zeros((4, 128, 512), dtype=np.float32)
    for r in range(4):
        kk = 128 * r + np.arange(128)[:, None]
        qq = np.arange(512)[None, :]
        cmask[r] = np.where(kk <= qq, 0.0, MASK_VAL)

    in_maps = []
    for c in range(N_CORES):
        heads = [HPC * c + h for h in range(HPC)]
        rows_q = np.concatenate([np.arange(h * HD, (h + 1) * HD) for h in heads])
        wq = W_qkv[rows_q, :]                  # [256, C]
        wk = W_qkv[C + rows_q, :]
        wv = W_qkv[2 * C + rows_q, :]
        wqk_t = _round_f32r(np.concatenate([wq, wk], axis=0).T)   # [C, 512]
        wv_t = _round_f32r(wv.T)                                  # [C, 256]

        ropeA = np.empty((HPC, HD, T), dtype=np.float32)
        ropeS = np.empty((HPC, HD, T), dtype=np.float32)
        for hl, h in enumerate(heads):
            g_h = g_qk[h * HD:(h + 1) * HD]                       # [HD]
            ropeA[hl] = cosT * g_h[:, None]
            sign = np.empty(HD, dtype=np.float32)
            sign[0::2] = -g_h[1::2]
            sign[1::2] = g_h[0::2]
            ropeS[hl] = sinT * sign[:, None]

        in_maps.append({
            "x": x_flat,
            "wqk_t": wqk_t,
            "wv_t": wv_t,
            "wp_t": wp_t,
            "ropeA": ropeA,
            "ropeS": ropeS,
            "rperm": _round_f32r(rperm),
            "ident": ident,
            "ones_col": ones_col,
            "ones_row": ones_row,
            "cmask": cmask,
        })
    return in_maps


def kernel(x: np.ndarray, W_qkv: np.ndarray, W_proj: np.ndarray,
           g_qk: np.ndarray) -> np.ndarray:
    global _NC_CACHE, _LAST_IN_MAPS
    if _NC_CACHE is None:
        _NC_CACHE = build_nc()
    nc = _NC_CACHE

    in_maps = prepare_in_maps(x, W_qkv, W_proj, g_qk)
    _LAST_IN_MAPS = in_maps
    res = run_bass_kernel_spmd(nc, in_maps, CORE_IDS)
    out_flat = np.empty((BT, C), dtype=np.float32)
    for c in range(N_CORES):
        out_flat[TOKS * c:TOKS * (c + 1), :] = res.results[c]["out"]
    return out_flat.reshape(B, T, C)


# revision 14
# speedup vs baseline: 1.1008x; 1.1008x over previous
"""Trainium2 Bass kernel for nn_CausalSelfAttention (B=2, T=2048, C=2048, H=16).

Sharding: tensor-parallel over heads (2 heads/core on 8 cores).
 - QKV projection computed per-core for its heads only (x replicated).
 - RMSNorm over full C needs cross-head sums of squares -> tiny AllReduce,
   overlapped with the scale-free part of RoPE.
 - Attention fully local per (batch, head) with a transposed, max-free softmax
   (scores bounded, so exp(tau*S - 10) never overflows; shift cancels),
   software-pipelined so PE never waits on the exp.
 - Output projection re-partitions y from head(channel)-sharding to
   token-row sharding via two AllToAlls (one per local head, the first
   overlapped with the second head's attention), then each core computes
   its 512 token rows against the full W_proj; host concatenates rows.

Matmuls run in float32r (fp32 rounded to 11-bit mantissa, full PE rate).
"""

import math

import numpy as np

import concourse.bass as bass
import concourse.mybir as mybir
import concourse.tile as tile
from concourse import bacc
from concourse.bass_utils import run_bass_kernel_spmd

N_CORES = 8
CORE_IDS = list(range(N_CORES))
B, T, C = 2, 2048, 2048
H, HD = 16, 128
HPC = H // N_CORES          # heads per core = 2
CPC = HPC * HD              # channels per core = 256
BT = B * T                  # 4096 token rows, batch-major
TOKS = BT // N_CORES        # 512 token rows per core for the projection
TC = 512                    # token-chunk for the QKV phase
TAU = 1.0 / math.sqrt(HD)
EPS = 1e-6
EXP_SHIFT = -10.0

f32 = mybir.dt.float32
f32r = mybir.dt.float32r


def _round_f32r(x: np.ndarray) -> np.ndarray:
    """Round fp32 to fp32r (11-bit mantissa, round-to-nearest-even)."""
    b = np.ascontiguousarray(x, dtype=np.float32).view(np.uint32)
    out = (b + np.uint32(0x7FF) + ((b >> np.uint32(12)) & np.uint32(1))) & np.uint32(
        0xFFFFF000
    )
    return out.view(np.float32)


def _rope_cos_sin():
    """cos/sin tables [T, HD], faithful to the reference construction."""
    i = np.arange(0, HD, 2, dtype=np.float32)
    pos = np.power(np.float32(10000.0), (-2.0 * i - 1.0).astype(np.float32) / np.float32(HD))
    token_seq = np.arange(T, dtype=np.float32)[:, None] * pos[None, :]
    ang = np.concatenate([token_seq, token_seq], axis=-1).astype(np.float32)
    return np.cos(ang), np.sin(ang)


def build_nc() -> bass.Bass:
    nc = bacc.Bacc(trn_type="TRN2", target_bir_lowering=False, num_devices=N_CORES)

    # ---- external inputs (per-core contents prepared on host) ----
    x_d = nc.dram_tensor("x", [BT, C], f32r, kind="ExternalInput")
    wqk_d = nc.dram_tensor("wqk_t", [C, 4 * HD], f32r, kind="ExternalInput")
    wv_d = nc.dram_tensor("wv_t", [C, CPC], f32r, kind="ExternalInput")
    wp_d = nc.dram_tensor("wp_t", [C, C], f32r, kind="ExternalInput")
    ra_d = nc.dram_tensor("ropeA", [HPC, HD, T], f32, kind="ExternalInput")
    rs_d = nc.dram_tensor("ropeS", [HPC, HD, T], f32, kind="ExternalInput")
    rp_d = nc.dram_tensor("rperm", [HD, HD], f32r, kind="ExternalInput")
    id_d = nc.dram_tensor("ident", [128, 128], f32r, kind="ExternalInput")
    oc_d = nc.dram_tensor("ones_col", [128, 1], f32r, kind="ExternalInput")
    or_d = nc.dram_tensor("ones_row", [1, 128], f32r, kind="ExternalInput")
    cm_d = nc.dram_tensor("cmask01", [4, 128, 512], f32, kind="ExternalInput")

    out_d = nc.dram_tensor("out", [TOKS, C], f32, kind="ExternalOutput")

    # ---- internal DRAM ----
    ssq_in = nc.dram_tensor("ssq_in", [1, 2 * BT], f32)
    ssq_out = nc.dram_tensor("ssq_out", [1, 2 * BT], f32, addr_space="Shared")
    srow_d = nc.dram_tensor("srow_bounce", [1, 2 * BT], f32r)
    vtok = nc.dram_tensor("vtok", [BT, CPC], f32r)
    a2a_in = [nc.dram_tensor(f"a2a_in{h}", [N_CORES, HD, TOKS], f32r)
              for h in range(HPC)]
    a2a_out = [nc.dram_tensor(f"a2a_out{h}", [N_CORES, HD, TOKS], f32r)
               for h in range(HPC)]

    NCH = C // 128  # 16 contraction chunks

    with tile.TileContext(nc) as tc:
        with tc.tile_pool(name="persist", bufs=1) as persist, \
             tc.tile_pool(name="consts", bufs=1) as consts:
            # persistent qT/kT storage: [q_h0, q_h1, k_h0, k_h1] x [128, BT]
            qk_sb = [persist.tile([128, BT], f32, tag=f"qk{i}", name=f"qk{i}")
                     for i in range(4)]

            ident = consts.tile([128, 128], f32r)
            nc.sync.dma_start(out=ident, in_=id_d[:, :])
            rperm = consts.tile([HD, HD], f32r)
            nc.sync.dma_start(out=rperm, in_=rp_d[:, :])
            ones_col = consts.tile([128, 1], f32r)
            nc.sync.dma_start(out=ones_col, in_=oc_d[:, :])
            ones_row = consts.tile([1, 128], f32r)
            nc.sync.dma_start(out=ones_row, in_=or_d[:, :])
            cmask = consts.tile([128, 4, 512], f32)
            nc.sync.dma_start(out=cmask, in_=cm_d.rearrange("r k t -> k r t"))
            eps_ap = consts.tile([128, 1], f32)
            nc.vector.memset(eps_ap, EPS)
            shift_ap = consts.tile([128, 1], f32)
            nc.vector.memset(shift_ap, EXP_SHIFT)

            # ================= QKV projection (+ per-chunk ssq partials) ======
            with tc.tile_pool(name="wqk", bufs=1) as wqkp, \
                 tc.tile_pool(name="wv", bufs=1) as wvp, \
                 tc.tile_pool(name="xin", bufs=2) as xinp, \
                 tc.tile_pool(name="xt", bufs=1) as xtp, \
                 tc.tile_pool(name="vev", bufs=3) as vevp, \
                 tc.tile_pool(name="sqt", bufs=2) as sqtp, \
                 tc.tile_pool(name="ssr", bufs=2) as ssrp, \
                 tc.tile_pool(name="ps_t", bufs=2, space="PSUM") as pst, \
                 tc.tile_pool(name="ps_mm", bufs=2, space="PSUM") as psmm, \
                 tc.tile_pool(name="ps_ss", bufs=2, space="PSUM") as psss:
                wqk_sb = wqkp.tile([128, NCH, 4 * HD], f32r)
                nc.sync.dma_start(out=wqk_sb, in_=wqk_d.rearrange("(c p) f -> p c f", p=128))
                wv_sb = wvp.tile([128, NCH, CPC], f32r)
                nc.sync.dma_start(out=wv_sb, in_=wv_d.rearrange("(c p) f -> p c f", p=128))

                for tcix in range(BT // TC):
                    r0 = tcix * TC
                    xt_t = xtp.tile([128, NCH, TC], f32r, tag="xt")
                    # load + transpose x chunk (f32r transposes, paired evicts)
                    for t4 in range(TC // 128):
                        x_t = xinp.tile([128, C], f32r, tag="xin")
                        nc.sync.dma_start(
                            out=x_t, in_=x_d[r0 + 128 * t4:r0 + 128 * (t4 + 1), :])
                        for cc2 in range(NCH // 2):
                            ps = pst.tile([128, 2, 128], f32r, tag="tp")
                            for u in range(2):
                                cc = 2 * cc2 + u
                                nc.tensor.transpose(
                                    ps[:, u, :], x_t[:, 128 * cc:128 * (cc + 1)], ident)
                            nc.any.tensor_copy(
                                out=xt_t[:, 2 * cc2:2 * cc2 + 2,
                                         128 * t4:128 * (t4 + 1)],
                                in_=ps)
                    # q/k blocks -> transposed layout [ch, tok]
                    for chb in range(4):
                        ps = psmm.tile([128, TC], f32, tag="qk")
                        for cc in range(NCH):
                            nc.tensor.matmul(
                                ps,
                                wqk_sb[:, cc, 128 * chb:128 * (chb + 1)],
                                xt_t[:, cc, :],
                                start=(cc == 0), stop=(cc == NCH - 1))
                        nc.any.tensor_copy(
                            out=qk_sb[chb][:, r0:r0 + TC].bitcast(f32r), in_=ps)
                    # v blocks -> token-major, spilled to DRAM
                    for t4 in range(TC // 128):
                        ps = psmm.tile([128, CPC], f32, tag="v")
                        for cc in range(NCH):
                            nc.tensor.matmul(
                                ps,
                                xt_t[:, cc, 128 * t4:128 * (t4 + 1)],
                                wv_sb[:, cc, :],
                                start=(cc == 0), stop=(cc == NCH - 1))
                        vev = vevp.tile([128, CPC], f32r, tag="vev")
                        nc.any.tensor_copy(out=vev, in_=ps)
                        nc.sync.dma_start(
                            out=vtok[r0 + 128 * t4:r0 + 128 * (t4 + 1), :], in_=vev)
                    # ssq partials for this chunk (q: chbs 0+1, k: chbs 2+3)
                    for ti in range(2):
                        ps = psss.tile([1, TC], f32, tag="ss")
                        for k in range(2):
                            chb = 2 * ti + k
                            sq = sqtp.tile([128, TC], f32r, tag="sq")
                            nc.vector.tensor_mul(
                                sq, qk_sb[chb][:, r0:r0 + TC],
                                qk_sb[chb][:, r0:r0 + TC])
                            nc.tensor.matmul(ps, ones_col, sq,
                                             start=(k == 0), stop=(k == 1))
                        ssr = ssrp.tile([1, TC], f32, tag="ssr")
                        nc.vector.tensor_copy(out=ssr, in_=ps)
                        nc.sync.dma_start(
                            out=ssq_in[0:1, BT * ti + r0: BT * ti + r0 + TC], in_=ssr)

            nc.gpsimd.collective_compute(
                "AllReduce", mybir.AluOpType.add,
                replica_groups=[CORE_IDS],
                ins=[ssq_in[:, :]], outs=[ssq_out[:, :]])

            # ============== RoPE (scale-free part, overlaps AllReduce) ========
            with tc.tile_pool(name="rope", bufs=1) as ropep, \
                 tc.tile_pool(name="rtmp", bufs=2) as rtmp, \
                 tc.tile_pool(name="nrm", bufs=1) as nrmp, \
                 tc.tile_pool(name="ps_r", bufs=2, space="PSUM") as psr:
                ra_sb = ropep.tile([128, HPC, T], f32)
                nc.sync.dma_start(out=ra_sb, in_=ra_d.rearrange("h d t -> d h t"))
                rs_sb = ropep.tile([128, HPC, T], f32)
                nc.sync.dma_start(out=rs_sb, in_=rs_d.rearrange("h d t -> d h t"))

                for chb in range(4):
                    h = chb % 2
                    for b in range(B):
                        for j4 in range(T // 512):
                            col = slice(b * T + 512 * j4, b * T + 512 * (j4 + 1))
                            tcol = slice(512 * j4, 512 * (j4 + 1))
                            ps = psr.tile([128, 512], f32, tag="rp")
                            nc.tensor.matmul(
                                ps, rperm, qk_sb[chb][:, col].bitcast(f32r),
                                start=True, stop=True)
                            t1 = rtmp.tile([128, 512], f32, tag="t1")
                            nc.vector.tensor_mul(t1, qk_sb[chb][:, col], ra_sb[:, h, tcol])
                            t2 = rtmp.tile([128, 512], f32, tag="t2")
                            nc.vector.tensor_mul(t2, ps, rs_sb[:, h, tcol])
                            nc.vector.tensor_add(qk_sb[chb][:, col].bitcast(f32r), t1, t2)

                # ---- norm scales (after AllReduce lands) + apply ----
                FR = 2 * BT // 128
                sq2d = nrmp.tile([128, FR], f32)
                nc.sync.dma_start(
                    out=sq2d, in_=ssq_out.rearrange("a (p f) -> (a p) f", p=128))
                sd2d = nrmp.tile([128, FR], f32)
                nc.scalar.activation(out=sd2d, in_=sq2d,
                                     func=mybir.ActivationFunctionType.Sqrt,
                                     bias=eps_ap, scale=1.0 / C)
                rc2d = nrmp.tile([128, FR], f32r)
                with nc.allow_low_precision(reason="f32r rounding of norm scale ok"):
                    nc.vector.reciprocal(out=rc2d, in_=sd2d)
                # reshape [128, FR] -> one row [1, 2*BT] via a DRAM bounce
                nc.sync.dma_start(
                    out=srow_d.rearrange("a (p f) -> (a p) f", p=128), in_=rc2d)
                s_row = nrmp.tile([1, 2 * BT], f32r)
                nc.sync.dma_start(out=s_row, in_=srow_d[:, :])

                for ti in range(2):
                    for c8 in range(BT // 512):
                        cols = slice(512 * c8, 512 * (c8 + 1))
                        ps = psr.tile([128, 512], f32, tag="rp")
                        nc.tensor.matmul(
                            ps, ones_row,
                            s_row[0:1, BT * ti + 512 * c8: BT * ti + 512 * (c8 + 1)],
                            start=True, stop=True)
                        sb = rtmp.tile([128, 512], f32, tag="t1")
                        nc.any.tensor_copy(out=sb, in_=ps)
                        for k in range(2):
                            chb = 2 * ti + k
                            nc.vector.tensor_mul(
                                out=qk_sb[chb][:, cols].bitcast(f32r),
                                in0=qk_sb[chb][:, cols], in1=sb)

            # ================= attention =================
            with tc.tile_pool(name="yt", bufs=1) as ytp, \
                 tc.tile_pool(name="vbh", bufs=2) as vbhp, \
                 tc.tile_pool(name="pt", bufs=4) as ptp, \
                 tc.tile_pool(name="att_sm", bufs=2) as attsm, \
                 tc.tile_pool(name="ps_s", bufs=3, space="PSUM") as pss, \
                 tc.tile_pool(name="ps_y", bufs=2, space="PSUM") as psy, \
                 tc.tile_pool(name="ps_d", bufs=2, space="PSUM") as psd:
                yt_sb = [ytp.tile([128, BT], f32, tag=f"yt{h}", name=f"yt{h}")
                         for h in range(HPC)]
                for h in range(HPC):
                    for b in range(B):
                        vb = vbhp.tile([128, T // 128, HD], f32r, tag="vbh")
                        nc.sync.dma_start(
                            out=vb,
                            in_=vtok[b * T:(b + 1) * T, HD * h:HD * (h + 1)].rearrange(
                                "(t p) d -> p t d", p=128))
                        for g in range(T // 512):
                            qcol = slice(b * T + 512 * g, b * T + 512 * (g + 1))
                            ps_yt = psy.tile([128, 512], f32, tag="y")
                            ps_dt = psd.tile([1, 512], f32, tag="d")
                            nj = 4 * g + 4
                            pts = [None] * nj
                            # software pipeline: S_j/exp_j run ahead of D/AV
                            for j in range(nj + 2):
                                if j < nj:
                                    kcol = slice(b * T + 128 * j, b * T + 128 * (j + 1))
                                    ps_st = pss.tile([128, 512], f32, tag="s")
                                    nc.tensor.matmul(
                                        ps_st,
                                        qk_sb[2 + h][:, kcol].bitcast(f32r),
                                        qk_sb[h][:, qcol].bitcast(f32r),
                                        start=True, stop=True)
                                    pt = ptp.tile([128, 512], f32r, tag="pt")
                                    nc.scalar.activation(
                                        out=pt, in_=ps_st,
                                        func=mybir.ActivationFunctionType.Exp,
                                        bias=shift_ap, scale=TAU)
                                    r = j - 4 * g
                                    if r >= 0:
                                        nc.vector.tensor_mul(
                                            out=pt.bitcast(f32r), in0=pt,
                                            in1=cmask[:, r, :])
                                    pts[j] = pt
                                jd = j - 2
                                if 0 <= jd < nj:
                                    nc.tensor.matmul(ps_dt, ones_col, pts[jd],
                                                     start=(jd == 0),
                                                     stop=(jd == nj - 1))
                                    nc.tensor.matmul(ps_yt, vb[:, jd, :], pts[jd],
                                                     start=(jd == 0),
                                                     stop=(jd == nj - 1))
                            d_sb = attsm.tile([1, 512], f32, tag="dsb")
                            nc.vector.tensor_copy(out=d_sb, in_=ps_dt)
                            r_sb = attsm.tile([1, 512], f32r, tag="rsb")
                            with nc.allow_low_precision(reason="f32r denom ok"):
                                nc.vector.reciprocal(out=r_sb, in_=d_sb)
                            ps_bt = pss.tile([128, 512], f32, tag="s")
                            nc.tensor.matmul(ps_bt, ones_row, r_sb[0:1, :],
                                             start=True, stop=True)
                            rb = attsm.tile([128, 512], f32, tag="rb")
                            nc.vector.tensor_copy(out=rb, in_=ps_bt)
                            nc.vector.tensor_mul(
                                out=yt_sb[h][:, qcol].bitcast(f32r),
                                in0=ps_yt, in1=rb)
                    # stage + launch this head's AllToAll (overlaps next head)
                    for p in range(N_CORES):
                        nc.sync.dma_start(
                            out=a2a_in[h][p, :, :],
                            in_=yt_sb[h][:, TOKS * p:TOKS * (p + 1)].bitcast(f32r))
                    nc.gpsimd.collective_compute(
                        "AllToAll", mybir.AluOpType.bypass,
                        replica_groups=[CORE_IDS],
                        ins=[a2a_in[h][:, :, :]], outs=[a2a_out[h][:, :, :]])

            # ================= output projection =================
            with tc.tile_pool(name="yf", bufs=1) as yfp, \
                 tc.tile_pool(name="wp", bufs=3) as wpp, \
                 tc.tile_pool(name="oev", bufs=3) as oevp, \
                 tc.tile_pool(name="ps_o", bufs=1, space="PSUM") as pso:
                yf_sb = yfp.tile([128, NCH, TOKS], f32r)
                for h in range(HPC):
                    for p in range(N_CORES):
                        nc.sync.dma_start(
                            out=yf_sb[:, HPC * p + h, :], in_=a2a_out[h][p, :, :])
                # contraction order: head-0 channels first (land earlier)
                cc_order = [HPC * p for p in range(N_CORES)] + \
                           [HPC * p + 1 for p in range(N_CORES)]
                for o4 in range(C // 512):
                    ps_list = [pso.tile([128, 512], f32, tag=f"o{t4}", name=f"o{o4}_{t4}")
                               for t4 in range(TOKS // 128)]
                    for ci, cc in enumerate(cc_order):
                        wp_sb = wpp.tile([128, 512], f32r, tag="wp")
                        nc.sync.dma_start(
                            out=wp_sb,
                            in_=wp_d[128 * cc:128 * (cc + 1), 512 * o4:512 * (o4 + 1)])
                        for t4 in range(TOKS // 128):
                            nc.tensor.matmul(
                                ps_list[t4],
                                yf_sb[:, cc, 128 * t4:128 * (t4 + 1)],
                                wp_sb,
                                start=(ci == 0), stop=(ci == NCH - 1))
                    for t4 in range(TOKS // 128):
                        o_sb = oevp.tile([128, 512], f32, tag="osb")
                        nc.any.tensor_copy(out=o_sb, in_=ps_list[t4])
                        nc.sync.dma_start(
                            out=out_d[128 * t4:128 * (t4 + 1), 512 * o4:512 * (o4 + 1)],
                            in_=o_sb)

    nc.finalize()
    return nc


_NC_CACHE = None
_LAST_IN_MAPS = None


def prepare_in_maps(x: np.ndarray, W_qkv: np.ndarray, W_proj: np.ndarray,
                    g_qk: np.ndarray) -> list:
    x = np.asarray(x, dtype=np.float32)
    W_qkv = np.asarray(W_qkv, dtype=np.float32)
    W_proj = np.asarray(W_proj, dtype=np.float32)
    g_qk = np.asarray(g_qk, dtype=np.float32)

    x_flat = _round_f32r(x.reshape(BT, C))
    wp_t = _round_f32r(W_proj.T)

    cos, sin = _rope_cos_sin()          # [T, HD]
    cosT, sinT = cos.T, sin.T           # [HD, T]

    ident = np.eye(128, dtype=np.float32)
    rperm = np.zeros((HD, HD), dtype=np.float32)
    for d in range(HD):
        rperm[d, d ^ 1] = 1.0
    ones_col = np.ones((128, 1), dtype=np.float32)
    ones_row = np.ones((1, 128), dtype=np.float32)

    cmask01 = np.zeros((4, 128, 512), dtype=np.float32)
    for r in range(4):
        kk = 128 * r + np.arange(128)[:, None]
        qq = np.arange(512)[None, :]
        cmask01[r] = (kk <= qq).astype(np.float32)

    in_maps = []
    for c in range(N_CORES):
        heads = [HPC * c + h for h in range(HPC)]
        rows_q = np.concatenate([np.arange(h * HD, (h + 1) * HD) for h in heads])
        wq = W_qkv[rows_q, :]                  # [256, C]
        wk = W_qkv[C + rows_q, :]
        wv = W_qkv[2 * C + rows_q, :]
        wqk_t = _round_f32r(np.concatenate([wq, wk], axis=0).T)   # [C, 512]
        wv_t = _round_f32r(wv.T)                                  # [C, 256]

        ropeA = np.empty((HPC, HD, T), dtype=np.float32)
        ropeS = np.empty((HPC, HD, T), dtype=np.float32)
        for hl, h in enumerate(heads):
            g_h = g_qk[h * HD:(h + 1) * HD]                       # [HD]
            ropeA[hl] = cosT * g_h[:, None]
            sign = np.empty(HD, dtype=np.float32)
            sign[0::2] = -g_h[1::2]
            sign[1::2] = g_h[0::2]
            ropeS[hl] = sinT * sign[:, None]

        in_maps.append({
            "x": x_flat,
            "wqk_t": wqk_t,
            "wv_t": wv_t,
            "wp_t": wp_t,
            "ropeA": ropeA,
            "ropeS": ropeS,
            "rperm": _round_f32r(rperm),
            "ident": _round_f32r(ident),
            "ones_col": ones_col,
            "ones_row": ones_row,
            "cmask01": cmask01,
        })
    return in_maps


def kernel(x: np.ndarray, W_qkv: np.ndarray, W_proj: np.ndarray,
           g_qk: np.ndarray) -> np.ndarray:
    global _NC_CACHE, _LAST_IN_MAPS
    if _NC_CACHE is None:
        _NC_CACHE = build_nc()
    nc = _NC_CACHE

    in_maps = prepare_in_maps(x, W_qkv, W_proj, g_qk)
    _LAST_IN_MAPS = in_maps
    res = run_bass_kernel_spmd(nc, in_maps, CORE_IDS)
    out_flat = np.empty((BT, C), dtype=np.float32)
    for c in range(N_CORES):
        out_flat[TOKS * c:TOKS * (c + 1), :] = res.results[c]["out"]
    return out_flat.reshape(B, T, C)


# revision 17
# speedup vs baseline: 1.2353x; 1.1222x over previous
"""Trainium2 Bass kernel for nn_CausalSelfAttention (B=2, T=2048, C=2048, H=16).

Sharding: tensor-parallel over heads (2 heads/core on 8 cores).
 - QKV projection computed per-core for its heads only (x replicated).
 - RMSNorm over full C needs cross-head sums of squares -> tiny AllReduce,
   overlapped with the scale-free part of RoPE.
 - Attention fully local per (batch, head) with a transposed, max-free softmax
   (scores bounded, so exp(tau*S - 10) never overflows; shift cancels),
   software-pipelined so PE never waits on the exp.
 - Output projection re-partitions y from head(channel)-sharding to
   token-row sharding via two AllToAlls (one per local head, the first
   overlapped with the second head's attention), then each core computes
   its 512 token rows against the full W_proj; host concatenates rows.

Matmuls run in float32r (fp32 rounded to 11-bit mantissa, full PE rate).
"""

import math

import numpy as np

import concourse.bass as bass
import concourse.mybir as mybir
import concourse.tile as tile
from concourse import bacc
from concourse.bass_utils import run_bass_kernel_spmd

N_CORES = 8
CORE_IDS = list(range(N_CORES))
B, T, C = 2, 2048, 2048
H, HD = 16, 128
HPC = H // N_CORES          # heads per core = 2
CPC = HPC * HD              # channels per core = 256
BT = B * T                  # 4096 token rows, batch-major
TOKS = BT // N_CORES        # 512 token rows per core for the projection
TC = 512                    # token-chunk for the QKV phase
TAU = 1.0 / math.sqrt(HD)
EPS = 1e-6
EXP_SHIFT = -10.0

f32 = mybir.dt.float32
f32r = mybir.dt.float32r


def _round_f32r(x: np.ndarray) -> np.ndarray:
    """Round fp32 to fp32r (11-bit mantissa, round-to-nearest-even)."""
    b = np.ascontiguousarray(x, dtype=np.float32).view(np.uint32)
    out = (b + np.uint32(0x7FF) + ((b >> np.uint32(12)) & np.uint32(1))) & np.uint32(
        0xFFFFF000
    )
    return out.view(np.float32)


def _rope_cos_sin():
    """cos/sin tables [T, HD], faithful to the reference construction."""
    i = np.arange(0, HD, 2, dtype=np.float32)
    pos = np.power(np.float32(10000.0), (-2.0 * i - 1.0).astype(np.float32) / np.float32(HD))
    token_seq = np.arange(T, dtype=np.float32)[:, None] * pos[None, :]
    ang = np.concatenate([token_seq, token_seq], axis=-1).astype(np.float32)
    return np.cos(ang), np.sin(ang)


def build_nc() -> bass.Bass:
    nc = bacc.Bacc(trn_type="TRN2", target_bir_lowering=False, num_devices=N_CORES)

    # ---- external inputs (per-core contents prepared on host) ----
    x_d = nc.dram_tensor("x", [BT, C], f32r, kind="ExternalInput")
    wqk_d = nc.dram_tensor("wqk_t", [C, 4 * HD], f32r, kind="ExternalInput")
    wv_d = nc.dram_tensor("wv_t", [C, CPC], f32r, kind="ExternalInput")
    wp_d = nc.dram_tensor("wp_t", [C, C], f32r, kind="ExternalInput")
    ra_d = nc.dram_tensor("ropeA", [HPC, HD, T], f32, kind="ExternalInput")
    rs_d = nc.dram_tensor("ropeS", [HPC, HD, T], f32, kind="ExternalInput")
    rp_d = nc.dram_tensor("rperm", [HD, HD], f32r, kind="ExternalInput")
    id_d = nc.dram_tensor("ident", [128, 128], f32r, kind="ExternalInput")
    oc_d = nc.dram_tensor("ones_col", [128, 1], f32r, kind="ExternalInput")
    or_d = nc.dram_tensor("ones_row", [1, 128], f32r, kind="ExternalInput")
    cm_d = nc.dram_tensor("cmask01", [4, 128, 512], f32, kind="ExternalInput")

    out_d = nc.dram_tensor("out", [TOKS, C], f32, kind="ExternalOutput")

    # ---- internal DRAM ----
    ssq_in = nc.dram_tensor("ssq_in", [1, 2 * BT], f32)
    ssq_out = nc.dram_tensor("ssq_out", [1, 2 * BT], f32, addr_space="Shared")
    srow_d = nc.dram_tensor("srow_bounce", [1, 2 * BT], f32r)
    vtok = nc.dram_tensor("vtok", [BT, CPC], f32r)
    a2a_in = [nc.dram_tensor(f"a2a_in{h}", [N_CORES, HD, TOKS], f32r)
              for h in range(HPC)]
    a2a_out = [nc.dram_tensor(f"a2a_out{h}", [N_CORES, HD, TOKS], f32r)
               for h in range(HPC)]

    NCH = C // 128  # 16 contraction chunks

    with tile.TileContext(nc) as tc:
        with tc.tile_pool(name="persist", bufs=1) as persist, \
             tc.tile_pool(name="consts", bufs=1) as consts:
            # persistent qT/kT storage: [q_h0, q_h1, k_h0, k_h1] x [128, BT]
            qk_sb = [persist.tile([128, BT], f32, tag=f"qk{i}", name=f"qk{i}")
                     for i in range(4)]

            ident = consts.tile([128, 128], f32r)
            nc.sync.dma_start(out=ident, in_=id_d[:, :])
            rperm = consts.tile([HD, HD], f32r)
            nc.sync.dma_start(out=rperm, in_=rp_d[:, :])
            ones_col = consts.tile([128, 1], f32r)
            nc.sync.dma_start(out=ones_col, in_=oc_d[:, :])
            ones_row = consts.tile([1, 128], f32r)
            nc.sync.dma_start(out=ones_row, in_=or_d[:, :])
            eps_ap = consts.tile([128, 1], f32)
            nc.vector.memset(eps_ap, EPS)
            shift_ap = consts.tile([128, 1], f32)
            nc.vector.memset(shift_ap, EXP_SHIFT)

            # ================= QKV projection (+ per-chunk ssq partials) ======
            with tc.tile_pool(name="wqk", bufs=1) as wqkp, \
                 tc.tile_pool(name="wv", bufs=1) as wvp, \
                 tc.tile_pool(name="xin", bufs=2) as xinp, \
                 tc.tile_pool(name="xt", bufs=1) as xtp, \
                 tc.tile_pool(name="vev", bufs=2) as vevp, \
                 tc.tile_pool(name="sqt", bufs=1) as sqtp, \
                 tc.tile_pool(name="ssr", bufs=2) as ssrp, \
                 tc.tile_pool(name="rope", bufs=1) as ropep, \
                 tc.tile_pool(name="rtmp", bufs=1) as rtmp, \
                 tc.tile_pool(name="ps_r", bufs=2, space="PSUM") as psr, \
                 tc.tile_pool(name="ps_t", bufs=2, space="PSUM") as pst, \
                 tc.tile_pool(name="ps_mm", bufs=2, space="PSUM") as psmm, \
                 tc.tile_pool(name="ps_v", bufs=1, space="PSUM") as psv_pool, \
                 tc.tile_pool(name="ps_ss", bufs=1, space="PSUM") as psss:
                ra_sb = ropep.tile([128, HPC, T], f32)
                nc.sync.dma_start(out=ra_sb, in_=ra_d.rearrange("h d t -> d h t"))
                rs_sb = ropep.tile([128, HPC, T], f32)
                nc.sync.dma_start(out=rs_sb, in_=rs_d.rearrange("h d t -> d h t"))
                wqk_sb = wqkp.tile([128, NCH, 4 * HD], f32r)
                nc.sync.dma_start(out=wqk_sb, in_=wqk_d.rearrange("(c p) f -> p c f", p=128))
                wv_sb = wvp.tile([128, NCH, CPC], f32r)
                nc.sync.dma_start(out=wv_sb, in_=wv_d.rearrange("(c p) f -> p c f", p=128))

                for tcix in range(BT // TC):
                    r0 = tcix * TC
                    xt_t = xtp.tile([128, NCH, TC], f32r, tag="xt")
                    # load + transpose x chunk (f32r transposes, paired evicts)
                    for t4 in range(TC // 128):
                        x_t = xinp.tile([128, C], f32r, tag="xin")
                        nc.sync.dma_start(
                            out=x_t, in_=x_d[r0 + 128 * t4:r0 + 128 * (t4 + 1), :])
                        for cc4 in range(NCH // 4):
                            ps = pst.tile([128, 4, 128], f32r, tag="tp")
                            for u in range(4):
                                cc = 4 * cc4 + u
                                nc.tensor.transpose(
                                    ps[:, u, :], x_t[:, 128 * cc:128 * (cc + 1)], ident)
                            nc.any.tensor_copy(
                                out=xt_t[:, 4 * cc4:4 * cc4 + 4,
                                         128 * t4:128 * (t4 + 1)],
                                in_=ps)
                    # q/k blocks -> transposed layout [ch, tok]
                    for chb in range(4):
                        ps = psmm.tile([128, TC], f32, tag="qk")
                        for cc in range(NCH):
                            nc.tensor.matmul(
                                ps,
                                wqk_sb[:, cc, 128 * chb:128 * (chb + 1)],
                                xt_t[:, cc, :],
                                start=(cc == 0), stop=(cc == NCH - 1))
                        nc.any.tensor_copy(
                            out=qk_sb[chb][:, r0:r0 + TC].bitcast(f32r), in_=ps)
                    # v blocks -> token-major, spilled to DRAM
                    for t4 in range(TC // 128):
                        ps = psv_pool.tile([128, CPC], f32, tag="v")
                        for cc in range(NCH):
                            nc.tensor.matmul(
                                ps,
                                xt_t[:, cc, 128 * t4:128 * (t4 + 1)],
                                wv_sb[:, cc, :],
                                start=(cc == 0), stop=(cc == NCH - 1))
                        vev = vevp.tile([128, CPC], f32r, tag="vev")
                        nc.any.tensor_copy(out=vev, in_=ps)
                        nc.sync.dma_start(
                            out=vtok[r0 + 128 * t4:r0 + 128 * (t4 + 1), :], in_=vev)
                    # ssq partials for this chunk (q: chbs 0+1, k: chbs 2+3)
                    for ti in range(2):
                        ps = psss.tile([1, TC], f32, tag="ss")
                        for k in range(2):
                            chb = 2 * ti + k
                            sq = sqtp.tile([128, TC], f32r, tag="sq")
                            nc.scalar.square(out=sq, in_=qk_sb[chb][:, r0:r0 + TC])
                            nc.tensor.matmul(ps, ones_col, sq,
                                             start=(k == 0), stop=(k == 1))
                        ssr = ssrp.tile([1, TC], f32, tag="ssr")
                        nc.vector.tensor_copy(out=ssr, in_=ps)
                        nc.sync.dma_start(
                            out=ssq_in[0:1, BT * ti + r0: BT * ti + r0 + TC], in_=ssr)
                    # RoPE (scale-free) for this chunk, overlaps later chunks
                    for chb in range(4):
                        h = chb % 2
                        b = r0 // T
                        col = slice(r0, r0 + TC)
                        tcol = slice(r0 - b * T, r0 - b * T + TC)
                        psv = psr.tile([128, 512], f32, tag="rp")
                        nc.tensor.matmul(
                            psv, rperm, qk_sb[chb][:, col].bitcast(f32r),
                            start=True, stop=True)
                        t1 = rtmp.tile([128, 512], f32, tag="t1")
                        nc.vector.tensor_mul(t1, qk_sb[chb][:, col], ra_sb[:, h, tcol])
                        t2 = rtmp.tile([128, 512], f32, tag="t2")
                        nc.vector.tensor_mul(t2, psv, rs_sb[:, h, tcol])
                        nc.vector.tensor_add(
                            qk_sb[chb][:, col].bitcast(f32r), t1, t2)

            nc.gpsimd.collective_compute(
                "AllReduce", mybir.AluOpType.add,
                replica_groups=[CORE_IDS],
                ins=[ssq_in[:, :]], outs=[ssq_out[:, :]])

            # ============== norm scales (after AllReduce lands) + apply =======
            with tc.tile_pool(name="rtmp", bufs=2) as rtmp, \
                 tc.tile_pool(name="nrm", bufs=1) as nrmp, \
                 tc.tile_pool(name="ps_r", bufs=2, space="PSUM") as psr:
                FR = 2 * BT // 128
                sq2d = nrmp.tile([128, FR], f32)
                nc.sync.dma_start(
                    out=sq2d, in_=ssq_out.rearrange("a (p f) -> (a p) f", p=128))
                sd2d = nrmp.tile([128, FR], f32)
                nc.scalar.activation(out=sd2d, in_=sq2d,
                                     func=mybir.ActivationFunctionType.Sqrt,
                                     bias=eps_ap, scale=1.0 / C)
                rc2d = nrmp.tile([128, FR], f32r)
                with nc.allow_low_precision(reason="f32r rounding of norm scale ok"):
                    nc.vector.reciprocal(out=rc2d, in_=sd2d)
                # reshape [128, FR] -> one row [1, 2*BT] via a DRAM bounce
                nc.sync.dma_start(
                    out=srow_d.rearrange("a (p f) -> (a p) f", p=128), in_=rc2d)
                s_row = nrmp.tile([1, 2 * BT], f32r)
                nc.sync.dma_start(out=s_row, in_=srow_d[:, :])

                for ti in range(2):
                    for c8 in range(BT // 512):
                        cols = slice(512 * c8, 512 * (c8 + 1))
                        ps = psr.tile([128, 512], f32, tag="rp")
                        nc.tensor.matmul(
                            ps, ones_row,
                            s_row[0:1, BT * ti + 512 * c8: BT * ti + 512 * (c8 + 1)],
                            start=True, stop=True)
                        sb = rtmp.tile([128, 512], f32, tag="t1")
                        nc.any.tensor_copy(out=sb, in_=ps)
                        for k in range(2):
                            chb = 2 * ti + k
                            nc.vector.tensor_mul(
                                out=qk_sb[chb][:, cols].bitcast(f32r),
                                in0=qk_sb[chb][:, cols], in1=sb)

            # ================= attention =================
            with tc.tile_pool(name="yt", bufs=1) as ytp, \
                 tc.tile_pool(name="vbh", bufs=2) as vbhp, \
                 tc.tile_pool(name="pt", bufs=4) as ptp, \
                 tc.tile_pool(name="att_sm", bufs=2) as attsm, \
                 tc.tile_pool(name="ps_s", bufs=3, space="PSUM") as pss, \
                 tc.tile_pool(name="ps_y", bufs=2, space="PSUM") as psy, \
                 tc.tile_pool(name="ps_d", bufs=2, space="PSUM") as psd, \
                 tc.tile_pool(name="cmaskp", bufs=1) as cmaskp:
                cmask = cmaskp.tile([128, 4, 512], f32)
                nc.sync.dma_start(out=cmask, in_=cm_d.rearrange("r k t -> k r t"))
                yt_sb = [ytp.tile([128, BT], f32, tag=f"yt{h}", name=f"yt{h}")
                         for h in range(HPC)]
                for h in range(HPC):
                    for b in range(B):
                        vb = vbhp.tile([128, T // 128, HD], f32r, tag="vbh")
                        nc.sync.dma_start(
                            out=vb,
                            in_=vtok[b * T:(b + 1) * T, HD * h:HD * (h + 1)].rearrange(
                                "(t p) d -> p t d", p=128))
                        for g in range(T // 512):
                            qcol = slice(b * T + 512 * g, b * T + 512 * (g + 1))
                            ps_yt = psy.tile([128, 512], f32, tag="y")
                            ps_dt = psd.tile([1, 512], f32, tag="d")
                            nj = 4 * g + 4
                            pts = [None] * nj
                            # software pipeline: S_j/exp_j run ahead of D/AV
                            for j in range(nj + 2):
                                if j < nj:
                                    kcol = slice(b * T + 128 * j, b * T + 128 * (j + 1))
                                    ps_st = pss.tile([128, 512], f32, tag="s")
                                    nc.tensor.matmul(
                                        ps_st,
                                        qk_sb[2 + h][:, kcol].bitcast(f32r),
                                        qk_sb[h][:, qcol].bitcast(f32r),
                                        start=True, stop=True)
                                    pt = ptp.tile([128, 512], f32r, tag="pt")
                                    nc.scalar.activation(
                                        out=pt, in_=ps_st,
                                        func=mybir.ActivationFunctionType.Exp,
                                        bias=shift_ap, scale=TAU)
                                    r = j - 4 * g
                                    if r >= 0:
                                        nc.vector.tensor_mul(
                                            out=pt.bitcast(f32r), in0=pt,
                                            in1=cmask[:, r, :])
                                    pts[j] = pt
                                jd = j - 2
                                if 0 <= jd < nj:
                                    nc.tensor.matmul(ps_dt, ones_col, pts[jd],
                                                     start=(jd == 0),
                                                     stop=(jd == nj - 1))
                                    nc.tensor.matmul(ps_yt, vb[:, jd, :], pts[jd],
                                                     start=(jd == 0),
                                                     stop=(jd == nj - 1))
                            d_sb = attsm.tile([1, 512], f32, tag="dsb")
                            nc.vector.tensor_copy(out=d_sb, in_=ps_dt)
                            db = attsm.tile([128, 512], f32, tag="db")
                            nc.gpsimd.partition_broadcast(db, d_sb, channels=128)
                            rb = attsm.tile([128, 512], f32r, tag="rb")
                            with nc.allow_low_precision(reason="f32r denom ok"):
                                nc.vector.reciprocal(out=rb, in_=db)
                            nc.vector.tensor_mul(
                                out=yt_sb[h][:, qcol].bitcast(f32r),
                                in0=ps_yt, in1=rb)
                    # stage + launch this head's AllToAll (overlaps next head)
                    for p in range(N_CORES):
                        nc.sync.dma_start(
                            out=a2a_in[h][p, :, :],
                            in_=yt_sb[h][:, TOKS * p:TOKS * (p + 1)].bitcast(f32r))
                    nc.gpsimd.collective_compute(
                        "AllToAll", mybir.AluOpType.bypass,
                        replica_groups=[CORE_IDS],
                        ins=[a2a_in[h][:, :, :]], outs=[a2a_out[h][:, :, :]])

            # ================= output projection =================
            with tc.tile_pool(name="yf", bufs=1) as yfp, \
                 tc.tile_pool(name="wp", bufs=3) as wpp, \
                 tc.tile_pool(name="oev", bufs=3) as oevp, \
                 tc.tile_pool(name="ps_o", bufs=2, space="PSUM") as pso:
                yf_sb = yfp.tile([128, NCH, TOKS], f32r)
                for h in range(HPC):
                    for p in range(N_CORES):
                        nc.sync.dma_start(
                            out=yf_sb[:, HPC * p + h, :], in_=a2a_out[h][p, :, :])
                # contraction order: head-0 channels first (land earlier)
                cc_order = [HPC * p for p in range(N_CORES)] + \
                           [HPC * p + 1 for p in range(N_CORES)]
                for o4 in range(C // 512):
                    ps_list = [pso.tile([128, 512], f32, tag=f"o{t4}", name=f"o{o4}_{t4}")
                               for t4 in range(TOKS // 128)]
                    for ci, cc in enumerate(cc_order):
                        wp_sb = wpp.tile([128, 512], f32r, tag="wp")
                        nc.sync.dma_start(
                            out=wp_sb,
                            in_=wp_d[128 * cc:128 * (cc + 1), 512 * o4:512 * (o4 + 1)])
                        for t4 in range(TOKS // 128):
                            nc.tensor.matmul(
                                ps_list[t4],
                                yf_sb[:, cc, 128 * t4:128 * (t4 + 1)],
                                wp_sb,
                                start=(ci == 0), stop=(ci == NCH - 1))
                    for t4 in range(TOKS // 128):
                        o_sb = oevp.tile([128, 512], f32, tag="osb")
                        nc.any.tensor_copy(out=o_sb, in_=ps_list[t4])
                        nc.sync.dma_start(
                            out=out_d[128 * t4:128 * (t4 + 1), 512 * o4:512 * (o4 + 1)],
                            in_=o_sb)

    nc.finalize()
    return nc


_NC_CACHE = None
_LAST_IN_MAPS = None


def prepare_in_maps(x: np.ndarray, W_qkv: np.ndarray, W_proj: np.ndarray,
                    g_qk: np.ndarray) -> list:
    x = np.asarray(x, dtype=np.float32)
    W_qkv = np.asarray(W_qkv, dtype=np.float32)
    W_proj = np.asarray(W_proj, dtype=np.float32)
    g_qk = np.asarray(g_qk, dtype=np.float32)

    x_flat = _round_f32r(x.reshape(BT, C))
    wp_t = _round_f32r(W_proj.T)

    cos, sin = _rope_cos_sin()          # [T, HD]
    cosT, sinT = cos.T, sin.T           # [HD, T]

    ident = np.eye(128, dtype=np.float32)
    rperm = np.zeros((HD, HD), dtype=np.float32)
    for d in range(HD):
        rperm[d, d ^ 1] = 1.0
    ones_col = np.ones((128, 1), dtype=np.float32)
    ones_row = np.ones((1, 128), dtype=np.float32)

    cmask01 = np.zeros((4, 128, 512), dtype=np.float32)
    for r in range(4):
        kk = 128 * r + np.arange(128)[:, None]
        qq = np.arange(512)[None, :]
        cmask01[r] = (kk <= qq).astype(np.float32)

    in_maps = []
    for c in range(N_CORES):
        heads = [HPC * c + h for h in range(HPC)]
        rows_q = np.concatenate([np.arange(h * HD, (h + 1) * HD) for h in heads])
        wq = W_qkv[rows_q, :]                  # [256, C]
        wk = W_qkv[C + rows_q, :]
        wv = W_qkv[2 * C + rows_q, :]
        wqk_t = _round_f32r(np.concatenate([wq, wk], axis=0).T)   # [C, 512]
        wv_t = _round_f32r(wv.T)                                  # [C, 256]

        ropeA = np.empty((HPC, HD, T), dtype=np.float32)
        ropeS = np.empty((HPC, HD, T), dtype=np.float32)
        for hl, h in enumerate(heads):
            g_h = g_qk[h * HD:(h + 1) * HD]                       # [HD]
            ropeA[hl] = cosT * g_h[:, None]
            sign = np.empty(HD, dtype=np.float32)
            sign[0::2] = -g_h[1::2]
            sign[1::2] = g_h[0::2]
            ropeS[hl] = sinT * sign[:, None]

        in_maps.append({
            "x": x_flat,
            "wqk_t": wqk_t,
            "wv_t": wv_t,
            "wp_t": wp_t,
            "ropeA": ropeA,
            "ropeS": ropeS,
            "rperm": _round_f32r(rperm),
            "ident": _round_f32r(ident),
            "ones_col": ones_col,
            "ones_row": ones_row,
            "cmask01": cmask01,
        })
    return in_maps


def kernel(x: np.ndarray, W_qkv: np.ndarray, W_proj: np.ndarray,
           g_qk: np.ndarray) -> np.ndarray:
    global _NC_CACHE, _LAST_IN_MAPS
    if _NC_CACHE is None:
        _NC_CACHE = build_nc()
    nc = _NC_CACHE

    in_maps = prepare_in_maps(x, W_qkv, W_proj, g_qk)
    _LAST_IN_MAPS = in_maps
    res = run_bass_kernel_spmd(nc, in_maps, CORE_IDS)
    out_flat = np.empty((BT, C), dtype=np.float32)
    for c in range(N_CORES):
        out_flat[TOKS * c:TOKS * (c + 1), :] = res.results[c]["out"]
    return out_flat.reshape(B, T, C)


# revision 25
# speedup vs baseline: 1.2727x; 1.0303x over previous
"""Trainium2 Bass kernel for nn_CausalSelfAttention (B=2, T=2048, C=2048, H=16).

Sharding: tensor-parallel over heads (2 heads/core on 8 cores).
 - QKV projection computed per-core for its heads only (x replicated).
 - RMSNorm over full C needs cross-head sums of squares -> tiny AllReduce,
   overlapped with the scale-free part of RoPE.
 - Attention fully local per (batch, head) with a transposed, max-free softmax
   (scores bounded, so exp(tau*S - 10) never overflows; shift cancels),
   software-pipelined so PE never waits on the exp.
 - Output projection re-partitions y from head(channel)-sharding to
   token-row sharding via two AllToAlls (one per local head, the first
   overlapped with the second head's attention), then each core computes
   its 512 token rows against the full W_proj; host concatenates rows.

Matmuls run in float32r (fp32 rounded to 11-bit mantissa, full PE rate).
"""

import math
from contextlib import ExitStack

import numpy as np

import concourse.bass as bass
import concourse.mybir as mybir
import concourse.tile as tile
from concourse import bacc
from concourse.bass_utils import run_bass_kernel_spmd

N_CORES = 8
CORE_IDS = list(range(N_CORES))
B, T, C = 2, 2048, 2048
H, HD = 16, 128
HPC = H // N_CORES          # heads per core = 2
CPC = HPC * HD              # channels per core = 256
BT = B * T                  # 4096 token rows, batch-major
TOKS = BT // N_CORES        # 512 token rows per core for the projection
TC = 512                    # token-chunk for the QKV phase
TAU = 1.0 / math.sqrt(HD)
EPS = 1e-6
EXP_SHIFT = -10.0

f32 = mybir.dt.float32
f32r = mybir.dt.float32r


def _round_f32r(x: np.ndarray) -> np.ndarray:
    """Round fp32 to fp32r (11-bit mantissa, round-to-nearest-even)."""
    b = np.ascontiguousarray(x, dtype=np.float32).view(np.uint32)
    out = (b + np.uint32(0x7FF) + ((b >> np.uint32(12)) & np.uint32(1))) & np.uint32(
        0xFFFFF000
    )
    return out.view(np.float32)


def _rope_cos_sin():
    """cos/sin tables [T, HD], faithful to the reference construction."""
    i = np.arange(0, HD, 2, dtype=np.float32)
    pos = np.power(np.float32(10000.0), (-2.0 * i - 1.0).astype(np.float32) / np.float32(HD))
    token_seq = np.arange(T, dtype=np.float32)[:, None] * pos[None, :]
    ang = np.concatenate([token_seq, token_seq], axis=-1).astype(np.float32)
    return np.cos(ang), np.sin(ang)


def build_nc() -> bass.Bass:
    nc = bacc.Bacc(trn_type="TRN2", target_bir_lowering=False, num_devices=N_CORES)

    # ---- external inputs (per-core contents prepared on host) ----
    x_d = nc.dram_tensor("x", [BT, C], f32r, kind="ExternalInput")
    wqk_d = nc.dram_tensor("wqk_t", [C, 4 * HD], f32r, kind="ExternalInput")
    wv_d = nc.dram_tensor("wv_t", [C, CPC], f32r, kind="ExternalInput")
    wp_d = nc.dram_tensor("wp_t", [C, C], f32r, kind="ExternalInput")
    ra_d = nc.dram_tensor("ropeA", [HPC, HD, T], f32, kind="ExternalInput")
    rs_d = nc.dram_tensor("ropeS", [HPC, HD, T], f32, kind="ExternalInput")
    rp_d = nc.dram_tensor("rperm", [HD, HD], f32r, kind="ExternalInput")
    id_d = nc.dram_tensor("ident", [128, 128], f32r, kind="ExternalInput")
    oc_d = nc.dram_tensor("ones_col", [128, 1], f32r, kind="ExternalInput")
    or_d = nc.dram_tensor("ones_row", [1, 128], f32r, kind="ExternalInput")
    cm_d = nc.dram_tensor("cmask01", [4, 128, 512], f32, kind="ExternalInput")

    out_d = nc.dram_tensor("out", [TOKS, C], f32, kind="ExternalOutput")

    # ---- internal DRAM ----
    HB = BT // 2  # tokens per half (= batch boundary for B=2)
    ssq_in = [nc.dram_tensor(f"ssq_in{hf}", [1, 2 * HB], f32) for hf in range(2)]
    ssq_out = [nc.dram_tensor(f"ssq_out{hf}", [1, 2 * HB], f32, addr_space="Shared")
               for hf in range(2)]
    srow_d = [nc.dram_tensor(f"srow_bounce{hf}", [1, 2 * HB], f32r) for hf in range(2)]
    vtok = nc.dram_tensor("vtok", [BT, CPC], f32r)
    a2a_in = [nc.dram_tensor(f"a2a_in{h}", [N_CORES, HD, TOKS], f32r)
              for h in range(HPC)]
    a2a_out = [nc.dram_tensor(f"a2a_out{h}", [N_CORES, HD, TOKS], f32r)
               for h in range(HPC)]

    NCH = C // 128  # 16 contraction chunks

    with tile.TileContext(nc) as tc:
        with ExitStack() as outer_es:
            persist = outer_es.enter_context(tc.tile_pool(name="persist", bufs=1))
            yfp = outer_es.enter_context(tc.tile_pool(name="yf", bufs=1))
            consts = outer_es.enter_context(tc.tile_pool(name="consts", bufs=1))
            yf_sb = yfp.tile([128, NCH, TOKS], f32r)
            # persistent qT/kT storage: [q_h0, q_h1, k_h0, k_h1] x [128, BT]
            qk_sb = [persist.tile([128, BT], f32, tag=f"qk{i}", name=f"qk{i}")
                     for i in range(4)]

            ident = consts.tile([128, 128], f32r)
            nc.sync.dma_start(out=ident, in_=id_d[:, :])
            rperm = consts.tile([HD, HD], f32r)
            nc.sync.dma_start(out=rperm, in_=rp_d[:, :])
            ones_col = consts.tile([128, 1], f32r)
            nc.sync.dma_start(out=ones_col, in_=oc_d[:, :])
            ones_row = consts.tile([1, 128], f32r)
            nc.sync.dma_start(out=ones_row, in_=or_d[:, :])
            eps_ap = consts.tile([128, 1], f32)
            nc.vector.memset(eps_ap, EPS)
            shift_ap = consts.tile([128, 1], f32)
            nc.vector.memset(shift_ap, EXP_SHIFT)

            # ================= QKV projection (+ per-chunk ssq partials) ======
            with ExitStack() as qkv_es:
                wqkp = qkv_es.enter_context(tc.tile_pool(name="wqk", bufs=1))
                wvp = qkv_es.enter_context(tc.tile_pool(name="wv", bufs=1))
                xinp = qkv_es.enter_context(tc.tile_pool(name="xin", bufs=3))
                xtp = qkv_es.enter_context(tc.tile_pool(name="xt", bufs=1))
                vevp = qkv_es.enter_context(tc.tile_pool(name="vev", bufs=1))
                sqtp = qkv_es.enter_context(tc.tile_pool(name="sqt", bufs=1))
                ssrp = qkv_es.enter_context(tc.tile_pool(name="ssr", bufs=1))
                ropep = qkv_es.enter_context(tc.tile_pool(name="rope", bufs=1))
                psr = qkv_es.enter_context(tc.tile_pool(name="ps_r", bufs=2, space="PSUM"))
                pst = qkv_es.enter_context(tc.tile_pool(name="ps_t", bufs=2, space="PSUM"))
                psmm = qkv_es.enter_context(tc.tile_pool(name="ps_mm", bufs=2, space="PSUM"))
                psv_pool = qkv_es.enter_context(tc.tile_pool(name="ps_v", bufs=1, space="PSUM"))
                psss = qkv_es.enter_context(tc.tile_pool(name="ps_ss", bufs=1, space="PSUM"))
                # prefetch chunk-0 x tiles before the big weight/table loads
                x_pre = [xinp.tile([128, C // 2], f32r, tag="xin", name=f"xpre{t}")
                         for t in range(2)]
                for t in range(2):
                    nc.sync.dma_start(out=x_pre[t],
                                      in_=x_d[0:128, (C // 2) * t:(C // 2) * (t + 1)])
                wqk_sb = wqkp.tile([128, NCH, 4 * HD], f32r)
                nc.sync.dma_start(out=wqk_sb, in_=wqk_d.rearrange("(c p) f -> p c f", p=128))
                wv_sb = wvp.tile([128, NCH, CPC], f32r)
                nc.sync.dma_start(out=wv_sb, in_=wv_d.rearrange("(c p) f -> p c f", p=128))

                for tcix in range(BT // TC):
                    r0 = tcix * TC
                    xt_t = xtp.tile([128, NCH, TC], f32r, tag="xt")
                    # load + transpose x chunk (f32r transposes, paired evicts)
                    for t4 in range(TC // 128):
                        for xh in range(2):
                            if tcix == 0 and t4 == 0:
                                x_t = x_pre[xh]
                            else:
                                x_t = xinp.tile([128, C // 2], f32r, tag="xin")
                                nc.sync.dma_start(
                                    out=x_t,
                                    in_=x_d[r0 + 128 * t4:r0 + 128 * (t4 + 1),
                                            (C // 2) * xh:(C // 2) * (xh + 1)])
                            for cc4 in range(NCH // 4 // 2):
                                ps = pst.tile([128, 4, 128], f32r, tag="tp")
                                for u in range(4):
                                    cl = 8 * xh + 4 * cc4 + u
                                    nc.tensor.transpose(
                                        ps[:, u, :],
                                        x_t[:, 128 * (cl - 8 * xh):128 * (cl - 8 * xh + 1)],
                                        ident)
                                nc.any.tensor_copy(
                                    out=xt_t[:, 8 * xh + 4 * cc4:8 * xh + 4 * cc4 + 4,
                                             128 * t4:128 * (t4 + 1)],
                                    in_=ps)
                    # q/k blocks -> transposed layout [ch, tok]
                    for chb in range(4):
                        ps = psmm.tile([128, TC], f32, tag="qk")
                        for cc in range(NCH):
                            nc.tensor.matmul(
                                ps,
                                wqk_sb[:, cc, 128 * chb:128 * (chb + 1)],
                                xt_t[:, cc, :],
                                start=(cc == 0), stop=(cc == NCH - 1))
                        nc.any.tensor_copy(
                            out=qk_sb[chb][:, r0:r0 + TC].bitcast(f32r), in_=ps)
                    # v blocks -> token-major, spilled to DRAM
                    for t4 in range(TC // 128):
                        ps = psv_pool.tile([128, CPC], f32, tag="v")
                        for cc in range(NCH):
                            nc.tensor.matmul(
                                ps,
                                xt_t[:, cc, 128 * t4:128 * (t4 + 1)],
                                wv_sb[:, cc, :],
                                start=(cc == 0), stop=(cc == NCH - 1))
                        vev = vevp.tile([128, CPC], f32r, tag="vev")
                        nc.any.tensor_copy(out=vev, in_=ps)
                        nc.sync.dma_start(
                            out=vtok[r0 + 128 * t4:r0 + 128 * (t4 + 1), :], in_=vev)
                    # ssq partials for this chunk (q: chbs 0+1, k: chbs 2+3)
                    for ti in range(2):
                        ps = psss.tile([1, TC], f32, tag="ss")
                        for k in range(2):
                            chb = 2 * ti + k
                            sq = sqtp.tile([128, TC], f32r, tag="sq")
                            nc.scalar.square(out=sq, in_=qk_sb[chb][:, r0:r0 + TC])
                            nc.tensor.matmul(ps, ones_col, sq,
                                             start=(k == 0), stop=(k == 1))
                        ssr = ssrp.tile([1, TC], f32, tag="ssr")
                        nc.vector.tensor_copy(out=ssr, in_=ps)
                        hf = r0 // HB
                        r0h = r0 - hf * HB
                        nc.sync.dma_start(
                            out=ssq_in[hf][0:1, HB * ti + r0h: HB * ti + r0h + TC],
                            in_=ssr)
                    # RoPE (scale-free) for this chunk, overlaps later chunks
                    b = r0 // T
                    tcol_d = slice(r0 - b * T, r0 - b * T + TC)
                    ra_sb = ropep.tile([128, HPC, TC], f32, tag="ra")
                    nc.sync.dma_start(
                        out=ra_sb, in_=ra_d[:, :, tcol_d].rearrange("h d t -> d h t"))
                    rs_sb = ropep.tile([128, HPC, TC], f32, tag="rs")
                    nc.sync.dma_start(
                        out=rs_sb, in_=rs_d[:, :, tcol_d].rearrange("h d t -> d h t"))
                    for chb in range(4):
                        h = chb % 2
                        col = slice(r0, r0 + TC)
                        psv = psr.tile([128, 512], f32, tag="rp")
                        nc.tensor.matmul(
                            psv, rperm, qk_sb[chb][:, col].bitcast(f32r),
                            start=True, stop=True)
                        t1 = sqtp.tile([128, 512], f32, tag="t1")
                        nc.vector.tensor_mul(t1, qk_sb[chb][:, col], ra_sb[:, h, :])
                        # psv *= S  (in-place PSUM), then qk = t1 + psv
                        nc.vector.tensor_mul(psv, psv, rs_sb[:, h, :])
                        nc.vector.tensor_add(
                            qk_sb[chb][:, col].bitcast(f32r), t1, psv)

            # ============== norm scales (per half, overlapping QKV/attn) =====
            with ExitStack() as nrm_es:
                rtmp = nrm_es.enter_context(tc.tile_pool(name="rtmp2", bufs=2))
                nrmp = nrm_es.enter_context(tc.tile_pool(name="nrm", bufs=2))
                psr = nrm_es.enter_context(tc.tile_pool(name="ps_r2", bufs=2, space="PSUM"))
                FR = 2 * HB // 128
                for hf in range(2):
                    nc.gpsimd.collective_compute(
                        "AllReduce", mybir.AluOpType.add,
                        replica_groups=[CORE_IDS],
                        ins=[ssq_in[hf][:, :]], outs=[ssq_out[hf][:, :]])
                    sq2d = nrmp.tile([128, FR], f32, tag="sq2d")
                    nc.sync.dma_start(
                        out=sq2d,
                        in_=ssq_out[hf].rearrange("a (p f) -> (a p) f", p=128))
                    sd2d = nrmp.tile([128, FR], f32, tag="sd2d")
                    nc.scalar.activation(out=sd2d, in_=sq2d,
                                         func=mybir.ActivationFunctionType.Sqrt,
                                         bias=eps_ap, scale=1.0 / C)
                    rc2d = nrmp.tile([128, FR], f32r, tag="rc2d")
                    with nc.allow_low_precision(reason="f32r norm scale ok"):
                        nc.vector.reciprocal(out=rc2d, in_=sd2d)
                    # reshape [128, FR] -> one row via a DRAM bounce
                    nc.sync.dma_start(
                        out=srow_d[hf].rearrange("a (p f) -> (a p) f", p=128),
                        in_=rc2d)
                    s_row = nrmp.tile([1, 2 * HB], f32r, tag="srow")
                    nc.sync.dma_start(out=s_row, in_=srow_d[hf][:, :])

                    for ti in range(2):
                        for c8 in range(HB // 512):
                            gcols = slice(hf * HB + 512 * c8, hf * HB + 512 * (c8 + 1))
                            ps = psr.tile([128, 512], f32, tag="rp")
                            nc.tensor.matmul(
                                ps, ones_row,
                                s_row[0:1, HB * ti + 512 * c8: HB * ti + 512 * (c8 + 1)],
                                start=True, stop=True)
                            sb = rtmp.tile([128, 512], f32, tag="t1")
                            nc.any.tensor_copy(out=sb, in_=ps)
                            for k in range(2):
                                chb = 2 * ti + k
                                nc.vector.tensor_mul(
                                    out=qk_sb[chb][:, gcols].bitcast(f32r),
                                    in0=qk_sb[chb][:, gcols], in1=sb)

            # ================= attention =================
            with ExitStack() as att_es:
                ytp = att_es.enter_context(tc.tile_pool(name="yt", bufs=1))
                vbhp = att_es.enter_context(tc.tile_pool(name="vbh", bufs=2))
                ptp = att_es.enter_context(tc.tile_pool(name="pt", bufs=4))
                attsm = att_es.enter_context(tc.tile_pool(name="att_sm", bufs=2))
                pss = att_es.enter_context(tc.tile_pool(name="ps_s", bufs=3, space="PSUM"))
                psy = att_es.enter_context(tc.tile_pool(name="ps_y", bufs=2, space="PSUM"))
                psd = att_es.enter_context(tc.tile_pool(name="ps_d", bufs=2, space="PSUM"))
                cmaskp = att_es.enter_context(tc.tile_pool(name="cmaskp", bufs=1))
                cmask = cmaskp.tile([128, 4, 512], f32)
                nc.sync.dma_start(out=cmask, in_=cm_d.rearrange("r k t -> k r t"))
                yt_sb = [ytp.tile([128, BT], f32, tag=f"yt{h}", name=f"yt{h}")
                         for h in range(HPC)]
                for h in range(HPC):
                    for b in range(B):
                        vb = vbhp.tile([128, T // 128, HD], f32r, tag="vbh")
                        nc.sync.dma_start(
                            out=vb,
                            in_=vtok[b * T:(b + 1) * T, HD * h:HD * (h + 1)].rearrange(
                                "(t p) d -> p t d", p=128))
                        for g in range(T // 512):
                            qcol = slice(b * T + 512 * g, b * T + 512 * (g + 1))
                            ps_yt = psy.tile([128, 512], f32, tag="y")
                            ps_dt = psd.tile([1, 512], f32, tag="d")
                            nj = 4 * g + 4
                            pts = [None] * nj
                            # software pipeline: S_j/exp_j run ahead of D/AV
                            for j in range(nj + 2):
                                if j < nj:
                                    kcol = slice(b * T + 128 * j, b * T + 128 * (j + 1))
                                    ps_st = pss.tile([128, 512], f32, tag="s")
                                    nc.tensor.matmul(
                                        ps_st,
                                        qk_sb[2 + h][:, kcol].bitcast(f32r),
                                        qk_sb[h][:, qcol].bitcast(f32r),
                                        start=True, stop=True)
                                    pt = ptp.tile([128, 512], f32r, tag="pt")
                                    nc.scalar.activation(
                                        out=pt, in_=ps_st,
                                        func=mybir.ActivationFunctionType.Exp,
                                        bias=shift_ap, scale=TAU)
                                    r = j - 4 * g
                                    if r >= 0:
                                        nc.vector.tensor_mul(
                                            out=pt.bitcast(f32r), in0=pt,
                                            in1=cmask[:, r, :])
                                    pts[j] = pt
                                jd = j - 2
                                if 0 <= jd < nj:
                                    nc.tensor.matmul(ps_dt, ones_col, pts[jd],
                                                     start=(jd == 0),
                                                     stop=(jd == nj - 1))
                                    nc.tensor.matmul(ps_yt, vb[:, jd, :], pts[jd],
                                                     start=(jd == 0),
                                                     stop=(jd == nj - 1))
                            d_sb = attsm.tile([1, 512], f32, tag="dsb")
                            nc.vector.tensor_copy(out=d_sb, in_=ps_dt)
                            db = attsm.tile([128, 512], f32, tag="db")
                            nc.gpsimd.partition_broadcast(db, d_sb, channels=128)
                            rb = attsm.tile([128, 512], f32r, tag="rb")
                            with nc.allow_low_precision(reason="f32r denom ok"):
                                nc.vector.reciprocal(out=rb, in_=db)
                            nc.vector.tensor_mul(
                                out=yt_sb[h][:, qcol].bitcast(f32r),
                                in0=ps_yt, in1=rb)
                    # stage + launch this head's AllToAll (overlaps next head)
                    for p in range(N_CORES):
                        nc.sync.dma_start(
                            out=a2a_in[h][p, :, :],
                            in_=yt_sb[h][:, TOKS * p:TOKS * (p + 1)].bitcast(f32r))
                    nc.gpsimd.collective_compute(
                        "AllToAll", mybir.AluOpType.bypass,
                        replica_groups=[CORE_IDS],
                        ins=[a2a_in[h][:, :, :]], outs=[a2a_out[h][:, :, :]])
                    for p in range(N_CORES):
                        nc.sync.dma_start(
                            out=yf_sb[:, HPC * p + h, :], in_=a2a_out[h][p, :, :])

            # ================= output projection =================
            with ExitStack() as proj_es:
                wpp = proj_es.enter_context(tc.tile_pool(name="wp", bufs=6))
                oevp = proj_es.enter_context(tc.tile_pool(name="oev", bufs=3))
                pso = proj_es.enter_context(tc.tile_pool(name="ps_o", bufs=2, space="PSUM"))
                # contraction order: head-0 channels first (land earlier)
                cc_order = [HPC * p for p in range(N_CORES)] + \
                           [HPC * p + 1 for p in range(N_CORES)]
                for o4 in range(C // 512):
                    ps_list = [pso.tile([128, 512], f32, tag=f"o{t4}", name=f"o{o4}_{t4}")
                               for t4 in range(TOKS // 128)]
                    for ci, cc in enumerate(cc_order):
                        wp_sb = wpp.tile([128, 512], f32r, tag="wp")
                        nc.sync.dma_start(
                            out=wp_sb,
                            in_=wp_d[128 * cc:128 * (cc + 1), 512 * o4:512 * (o4 + 1)])
                        for t4 in range(TOKS // 128):
                            nc.tensor.matmul(
                                ps_list[t4],
                                yf_sb[:, cc, 128 * t4:128 * (t4 + 1)],
                                wp_sb,
                                start=(ci == 0), stop=(ci == NCH - 1))
                    for t4 in range(TOKS // 128):
                        o_sb = oevp.tile([128, 512], f32, tag="osb")
                        nc.any.tensor_copy(out=o_sb, in_=ps_list[t4])
                        nc.sync.dma_start(
                            out=out_d[128 * t4:128 * (t4 + 1), 512 * o4:512 * (o4 + 1)],
                            in_=o_sb)

    nc.finalize()
    return nc


_NC_CACHE = None
_LAST_IN_MAPS = None


def prepare_in_maps(x: np.ndarray, W_qkv: np.ndarray, W_proj: np.ndarray,
                    g_qk: np.ndarray) -> list:
    x = np.asarray(x, dtype=np.float32)
    W_qkv = np.asarray(W_qkv, dtype=np.float32)
    W_proj = np.asarray(W_proj, dtype=np.float32)
    g_qk = np.asarray(g_qk, dtype=np.float32)

    x_flat = _round_f32r(x.reshape(BT, C))
    wp_t = _round_f32r(W_proj.T)

    cos, sin = _rope_cos_sin()          # [T, HD]
    cosT, sinT = cos.T, sin.T           # [HD, T]

    ident = np.eye(128, dtype=np.float32)
    rperm = np.zeros((HD, HD), dtype=np.float32)
    for d in range(HD):
        rperm[d, d ^ 1] = 1.0
    ones_col = np.ones((128, 1), dtype=np.float32)
    ones_row = np.ones((1, 128), dtype=np.float32)

    cmask01 = np.zeros((4, 128, 512), dtype=np.float32)
    for r in range(4):
        kk = 128 * r + np.arange(128)[:, None]
        qq = np.arange(512)[None, :]
        cmask01[r] = (kk <= qq).astype(np.float32)

    in_maps = []
    for c in range(N_CORES):
        heads = [HPC * c + h for h in range(HPC)]
        rows_q = np.concatenate([np.arange(h * HD, (h + 1) * HD) for h in heads])
        wq = W_qkv[rows_q, :]                  # [256, C]
        wk = W_qkv[C + rows_q, :]
        wv = W_qkv[2 * C + rows_q, :]
        wqk_t = _round_f32r(np.concatenate([wq, wk], axis=0).T)   # [C, 512]
        wv_t = _round_f32r(wv.T)                                  # [C, 256]

        ropeA = np.empty((HPC, HD, T), dtype=np.float32)
        ropeS = np.empty((HPC, HD, T), dtype=np.float32)
        for hl, h in enumerate(heads):
            g_h = g_qk[h * HD:(h + 1) * HD]                       # [HD]
            ropeA[hl] = cosT * g_h[:, None]
            sign = np.empty(HD, dtype=np.float32)
            sign[0::2] = -g_h[1::2]
            sign[1::2] = g_h[0::2]
            ropeS[hl] = sinT * sign[:, None]

        in_maps.append({
            "x": x_flat,
            "wqk_t": wqk_t,
            "wv_t": wv_t,
            "wp_t": wp_t,
            "ropeA": ropeA,
            "ropeS": ropeS,
            "rperm": _round_f32r(rperm),
            "ident": _round_f32r(ident),
            "ones_col": ones_col,
            "ones_row": ones_row,
            "cmask01": cmask01,
        })
    return in_maps


def kernel(x: np.ndarray, W_qkv: np.ndarray, W_proj: np.ndarray,
           g_qk: np.ndarray) -> np.ndarray:
    global _NC_CACHE, _LAST_IN_MAPS
    if _NC_CACHE is None:
        _NC_CACHE = build_nc()
    nc = _NC_CACHE

    in_maps = prepare_in_maps(x, W_qkv, W_proj, g_qk)
    _LAST_IN_MAPS = in_maps
    res = run_bass_kernel_spmd(nc, in_maps, CORE_IDS)
    out_flat = np.empty((BT, C), dtype=np.float32)
    for c in range(N_CORES):
        out_flat[TOKS * c:TOKS * (c + 1), :] = res.results[c]["out"]
    return out_flat.reshape(B, T, C)
